# revision 5
# baseline (speedup 1.0000x reference)
"""DeepseekV3 MLA attention kernel for 8 Trainium2 NeuronCores.

Sharding: 2-way data-parallel over batch x 4-way tensor-parallel over heads.
Core c handles batch b = c // 4 and heads [4*(c%4) .. 4*(c%4)+4).
Each core computes the LoRA-A compression for its batch (replicated within
the batch group), head-sharded projections + RoPE + attention + its slice
of the w_o contraction; the host sums the 4 partial outputs per batch.

All matmuls run as float32r (full fp32 storage, fast PE mode).
"""

from contextlib import ExitStack
from dataclasses import dataclass

import numpy as np

import concourse.bacc as bacc
import concourse.mybir as mybir
import concourse.tile as tile

F32 = mybir.dt.float32
F32R = mybir.dt.float32r


@dataclass(frozen=True)
class Cfg:
    S: int = 2048          # sequence length (per batch)
    HID: int = 2048        # hidden dim
    QLR: int = 1536        # q lora rank
    KVLR: int = 512        # kv lora rank
    NH_G: int = 4          # heads per core
    DN: int = 128          # nope dim
    DR: int = 64           # rope dim
    DV: int = 128          # v head dim
    S1T: int = 256         # stage-1 s-block width
    S2T: int = 512         # stage-2 s-tile width
    QT: int = 512          # attention q-tile width

    @property
    def QCD(self):
        return self.QLR + self.KVLR

    @property
    def SCALE(self):
        return 1.0 / float(np.sqrt(self.DN + self.DR))


CFG = Cfg()


def r(ap):
    """bitcast an fp32 AP to float32r for fast PE mode."""
    return ap.bitcast(F32R)


def build_nc(C: Cfg):
    nc = bacc.Bacc("TRN2", target_bir_lowering=False, debug=False, num_devices=8)
    P = 128
    HO = C.HID // P            # h-chunks
    NQC = C.QCD // P           # qc+kvc row tiles
    NS1 = C.S // C.S1T         # stage-1 s blocks
    NS2 = C.S // C.S2T         # stage-2 s tiles
    QLC = C.QLR // P           # q lora chunks
    KVC = C.KVLR // P          # kv lora chunks
    NPAIR = C.NH_G // 2        # rope head pairs
    NQT = C.S // C.QT          # attention q tiles
    NDIAG = C.QT // P          # diagonal k-subtiles per q tile
    NVS = C.S // P             # v row blocks

    # ---- kernel I/O ----
    hT = nc.dram_tensor("hT", [C.HID, C.S], F32R, kind="ExternalInput").ap()
    w_a = nc.dram_tensor("w_a", [C.HID, C.QCD], F32R, kind="ExternalInput").ap()
    w_qbn = nc.dram_tensor("w_qbn", [C.QLR, C.NH_G * C.DN], F32R, kind="ExternalInput").ap()
    w_qbr = nc.dram_tensor("w_qbr", [C.QLR, C.NH_G * C.DR], F32R, kind="ExternalInput").ap()
    w_kbn = nc.dram_tensor("w_kbn", [C.KVLR, C.NH_G * C.DN], F32R, kind="ExternalInput").ap()
    w_kbr = nc.dram_tensor("w_kbr", [C.KVLR, C.NH_G * C.DR], F32R, kind="ExternalInput").ap()
    w_vb = nc.dram_tensor("w_vb", [C.KVLR, C.NH_G * C.DV], F32R, kind="ExternalInput").ap()
    w_ob = nc.dram_tensor("w_ob", [C.NH_G * C.DV, C.HID], F32R, kind="ExternalInput").ap()
    cos2 = nc.dram_tensor("cos2", [P, C.S], F32, kind="ExternalInput").ap()
    ssin2 = nc.dram_tensor("ssin2", [P, C.S], F32, kind="ExternalInput").ap()
    dmask = nc.dram_tensor("dmask", [C.QT, C.QT], F32, kind="ExternalInput").ap()
    outp = nc.dram_tensor("outp", [C.S, C.HID], F32, kind="ExternalOutput").ap()

    # ---- DRAM scratch ----
    qckv = nc.dram_tensor("qckv_scr", [C.QCD, C.S], F32R).ap()
    qTn_d = nc.dram_tensor("qTn_scr", [C.NH_G * C.DN, C.S], F32R).ap()
    qTr_d = nc.dram_tensor("qTr_scr", [C.NH_G * C.DR, C.S], F32R).ap()

    with tile.TileContext(nc) as tc:
        # =========== Phase 1: q_c/kv_c = hidden @ [w_q_a | w_kv_a] ==========
        # computed transposed: qckv[qc, s] with w_a as stationary, hT moving
        with ExitStack() as ctx:
            wa_pool = ctx.enter_context(tc.tile_pool(name="wa", bufs=1))
            ht_pool = ctx.enter_context(tc.tile_pool(name="ht", bufs=2))
            ev_pool = ctx.enter_context(tc.tile_pool(name="s1ev", bufs=4))
            ps_pool = ctx.enter_context(tc.tile_pool(name="s1ps", bufs=4, space="PSUM"))

            wa_sb = wa_pool.tile([P, HO, C.QCD], F32R)
            wa_r = w_a.rearrange("(ho hi) c -> hi ho c", hi=P)
            for qc in range(NQC):
                nc.sync.dma_start(
                    out=wa_sb[:, :, qc * P:(qc + 1) * P],
                    in_=wa_r[:, :, qc * P:(qc + 1) * P],
                )
            hT_r = hT.rearrange("(ho hi) s -> hi ho s", hi=P)
            for st in range(NS1):
                s0 = st * C.S1T
                ht_sb = ht_pool.tile([P, HO, C.S1T], F32R)
                nc.sync.dma_start(out=ht_sb[:], in_=hT_r[:, :, s0:s0 + C.S1T])
                for qc in range(NQC):
                    ps = ps_pool.tile([P, C.S1T], F32)
                    for h in range(HO):
                        nc.tensor.matmul(
                            ps[:],
                            (wa_sb[:, h, qc * P:(qc + 1) * P]),
                            (ht_sb[:, h, :]),
                            start=(h == 0), stop=(h == HO - 1),
                        )
                    ev = ev_pool.tile([P, C.S1T], F32R)
                    nc.vector.tensor_copy(ev[:], ps[:])
                    nc.sync.dma_start(
                        out=qckv[qc * P:(qc + 1) * P, s0:s0 + C.S1T], in_=ev[:]
                    )

        # rope tables + persistent attention operands
        persist = tc.tile_pool(name="persist", bufs=1)
        with persist as pp:
            cos_sb = pp.tile([P, C.S], F32)
            ssin_sb = pp.tile([P, C.S], F32)
            nc.sync.dma_start(out=cos_sb[:], in_=cos2)
            nc.sync.dma_start(out=ssin_sb[:], in_=ssin2)
            kTn_sb = pp.tile([P, C.NH_G, C.S], F32R)
            kTr_sb = pp.tile([P, NPAIR, C.S], F32R)
            v_sb = pp.tile([P, NVS, C.NH_G * C.DV], F32R)

            def rope_evict(ctx2, pool, rp_pool, ps, dst_ap, s0, slen):
                """psum [128, slen] of rope rows (head pair) -> roped into dst."""
                tmp = rp_pool.tile([P, slen], F32R, tag="rope_tmp")
                nc.vector.tensor_copy(tmp[:], ps[:])
                qs = rp_pool.tile([P, slen], F32R, tag="rope_qs")
                for g in range(4):
                    src = tmp[g * 32:(g + 1) * 32, :]
                    d0 = (g ^ 1) * 32
                    nc.sync.dma_start(out=qs[d0:d0 + 32, :], in_=src)
                m1 = rp_pool.tile([P, slen], F32R, tag="rope_m1")
                nc.vector.tensor_mul(m1[:], tmp[:], cos_sb[:, s0:s0 + slen])
                nc.vector.tensor_mul(qs[:], qs[:], ssin_sb[:, s0:s0 + slen])
                nc.vector.tensor_add(dst_ap, m1[:], qs[:])

            # ========= Phase 2a: qT projections (to DRAM, roped) =========
            with ExitStack() as ctx:
                wq_pool = ctx.enter_context(tc.tile_pool(name="wq", bufs=1))
                qc_pool = ctx.enter_context(tc.tile_pool(name="qcs", bufs=2))
                ev_pool = ctx.enter_context(tc.tile_pool(name="s2ev", bufs=4))
                rp_pool = ctx.enter_context(tc.tile_pool(name="s2rp", bufs=3))
                ps_pool = ctx.enter_context(tc.tile_pool(name="s2ps", bufs=4, space="PSUM"))

                wqn_sb = wq_pool.tile([P, QLC, C.NH_G * C.DN], F32R)
                wqr_sb = wq_pool.tile([P, QLC, C.NH_G * C.DR], F32R)
                nc.sync.dma_start(
                    out=wqn_sb[:], in_=w_qbn.rearrange("(co ci) m -> ci co m", ci=P))
                nc.sync.dma_start(
                    out=wqr_sb[:], in_=w_qbr.rearrange("(co ci) m -> ci co m", ci=P))
                qc_r = qckv[0:C.QLR, :].rearrange("(co ci) s -> ci co s", ci=P)
                for st in range(NS2):
                    s0 = st * C.S2T
                    qc_sb = qc_pool.tile([P, QLC, C.S2T], F32R)
                    nc.sync.dma_start(out=qc_sb[:], in_=qc_r[:, :, s0:s0 + C.S2T])
                    for h in range(C.NH_G):
                        ps = ps_pool.tile([P, C.S2T], F32)
                        for cch in range(QLC):
                            nc.tensor.matmul(
                                ps[:],
                                (wqn_sb[:, cch, h * C.DN:(h + 1) * C.DN]),
                                (qc_sb[:, cch, :]),
                                start=(cch == 0), stop=(cch == QLC - 1),
                            )
                        ev = ev_pool.tile([P, C.S2T], F32R)
                        nc.vector.tensor_copy(ev[:], ps[:])
                        nc.sync.dma_start(
                            out=qTn_d[h * C.DN:(h + 1) * C.DN, s0:s0 + C.S2T],
                            in_=ev[:])
                    for pr in range(NPAIR):
                        ps = ps_pool.tile([P, C.S2T], F32)
                        for cch in range(QLC):
                            nc.tensor.matmul(
                                ps[:],
                                (wqr_sb[:, cch, pr * P:(pr + 1) * P]),
                                (qc_sb[:, cch, :]),
                                start=(cch == 0), stop=(cch == QLC - 1),
                            )
                        ev = ev_pool.tile([P, C.S2T], F32R)
                        rope_evict(ctx, ev_pool, rp_pool, ps, ev[:], s0, C.S2T)
                        nc.sync.dma_start(
                            out=qTr_d[pr * P:(pr + 1) * P, s0:s0 + C.S2T], in_=ev[:])

            # ========= Phase 2b: kT / v projections (stay resident) =========
            with ExitStack() as ctx:
                wk_pool = ctx.enter_context(tc.tile_pool(name="wk", bufs=1))
                kv_pool = ctx.enter_context(tc.tile_pool(name="kvs", bufs=2))
                rp_pool = ctx.enter_context(tc.tile_pool(name="s2brp", bufs=3))
                ps_pool = ctx.enter_context(tc.tile_pool(name="s2bps", bufs=4, space="PSUM"))

                wkn_sb = wk_pool.tile([P, KVC, C.NH_G * C.DN], F32R)
                wkr_sb = wk_pool.tile([P, KVC, C.NH_G * C.DR], F32R)
                wv_sb = wk_pool.tile([P, KVC, C.NH_G * C.DV], F32R)
                nc.sync.dma_start(
                    out=wkn_sb[:], in_=w_kbn.rearrange("(co ci) m -> ci co m", ci=P))
                nc.sync.dma_start(
                    out=wkr_sb[:], in_=w_kbr.rearrange("(co ci) m -> ci co m", ci=P))
                nc.sync.dma_start(
                    out=wv_sb[:], in_=w_vb.rearrange("(co ci) m -> ci co m", ci=P))
                kv_r = qckv[C.QLR:C.QCD, :].rearrange("(co ci) s -> ci co s", ci=P)
                for st in range(NS2):
                    s0 = st * C.S2T
                    kv_sb = kv_pool.tile([P, KVC, C.S2T], F32R)
                    nc.sync.dma_start(out=kv_sb[:], in_=kv_r[:, :, s0:s0 + C.S2T])
                    for h in range(C.NH_G):
                        ps = ps_pool.tile([P, C.S2T], F32)
                        for cch in range(KVC):
                            nc.tensor.matmul(
                                ps[:],
                                (wkn_sb[:, cch, h * C.DN:(h + 1) * C.DN]),
                                (kv_sb[:, cch, :]),
                                start=(cch == 0), stop=(cch == KVC - 1),
                            )
                        nc.vector.tensor_copy(kTn_sb[:, h, s0:s0 + C.S2T], ps[:])
                    for pr in range(NPAIR):
                        ps = ps_pool.tile([P, C.S2T], F32)
                        for cch in range(KVC):
                            nc.tensor.matmul(
                                ps[:],
                                (wkr_sb[:, cch, pr * P:(pr + 1) * P]),
                                (kv_sb[:, cch, :]),
                                start=(cch == 0), stop=(cch == KVC - 1),
                            )
                        rope_evict(ctx, None, rp_pool, ps,
                                   kTr_sb[:, pr, s0:s0 + C.S2T], s0, C.S2T)
                    # v in natural [s, dv] orientation: lhsT = kv_cT slice
                    for ssub in range(C.S2T // P):
                        vs = (s0 + ssub * P) // P
                        ps = ps_pool.tile([P, C.NH_G * C.DV], F32)
                        for cch in range(KVC):
                            nc.tensor.matmul(
                                ps[:],
                                (kv_sb[:, cch, ssub * P:(ssub + 1) * P]),
                                (wv_sb[:, cch, :]),
                                start=(cch == 0), stop=(cch == KVC - 1),
                            )
                        nc.vector.tensor_copy(v_sb[:, vs, :], ps[:])

            # ================= Phase 3: attention + w_o =================
            with ExitStack() as ctx:
                const_pool = ctx.enter_context(tc.tile_pool(name="a_const", bufs=1))
                qn_pool = ctx.enter_context(tc.tile_pool(name="a_qn", bufs=2))
                qr_pool = ctx.enter_context(tc.tile_pool(name="a_qr", bufs=2))
                e_pool = ctx.enter_context(tc.tile_pool(name="a_e", bufs=6))
                d_pool = ctx.enter_context(tc.tile_pool(name="a_d", bufs=4))
                ao_pool = ctx.enter_context(tc.tile_pool(name="a_ao", bufs=2))
                oev_pool = ctx.enter_context(tc.tile_pool(name="a_oev", bufs=4))
                ps_s = ctx.enter_context(tc.tile_pool(name="a_pss", bufs=3, space="PSUM"))
                ps_d = ctx.enter_context(tc.tile_pool(name="a_psd", bufs=1, space="PSUM"))
                ps_o = ctx.enter_context(tc.tile_pool(name="a_pso", bufs=2, space="PSUM"))
                ps_w = ctx.enter_context(tc.tile_pool(name="a_psw", bufs=2, space="PSUM"))

                ones_f = const_pool.tile([P, P], F32)
                ones_sb = const_pool.tile([P, P], F32R)
                nc.vector.memset(ones_f[:], 1.0)
                nc.vector.tensor_copy(ones_sb[:], ones_f[:])
                dm_sb = const_pool.tile([P, NDIAG, C.QT], F32)
                nc.sync.dma_start(
                    out=dm_sb[:], in_=dmask.rearrange("(j ki) q -> ki j q", ki=P))
                wo_sb = const_pool.tile([P, C.NH_G, C.HID], F32R)
                nc.sync.dma_start(
                    out=wo_sb[:], in_=w_ob.rearrange("(h d) o -> d h o", d=P))

                for qt in range(NQT):
                    q0 = qt * C.QT
                    nkt = (qt + 1) * C.QT // P
                    ao_sb = ao_pool.tile([P, C.NH_G, C.QT], F32R, tag="ao")
                    for pr in range(NPAIR):
                        qr_sb = qr_pool.tile([P, C.QT], F32R, tag="qr")
                        nc.sync.dma_start(
                            out=qr_sb[:],
                            in_=qTr_d[pr * P:(pr + 1) * P, q0:q0 + C.QT])
                        for hh in range(2):
                            h = pr * 2 + hh
                            qn_sb = qn_pool.tile([P, C.QT], F32R, tag="qn")
                            nc.sync.dma_start(
                                out=qn_sb[:],
                                in_=qTn_d[h * C.DN:(h + 1) * C.DN, q0:q0 + C.QT])
                            rsl = slice(hh * C.DR, (hh + 1) * C.DR)
                            psd = ps_d.tile([P, C.QT], F32, tag="psd")
                            pso = ps_o.tile([P, C.QT], F32, tag="pso")
                            for kt in range(nkt):
                                k0 = kt * P
                                pss = ps_s.tile([P, C.QT], F32, tag="pss")
                                nc.tensor.matmul(
                                    pss[:],
                                    (kTn_sb[:, h, k0:k0 + P]),
                                    (qn_sb[:]),
                                    start=True, stop=False)
                                nc.tensor.matmul(
                                    pss[:],
                                    (kTr_sb[rsl, pr, k0:k0 + P]),
                                    (qr_sb[rsl, :]),
                                    start=False, stop=True)
                                e_sb = e_pool.tile([P, C.QT], F32R, tag="e")
                                nc.scalar.activation(
                                    e_sb[:], pss[:],
                                    mybir.ActivationFunctionType.Exp,
                                    scale=C.SCALE)
                                j = kt - qt * NDIAG
                                if j >= 0:
                                    nc.vector.tensor_mul(
                                        e_sb[:], e_sb[:], dm_sb[:, j, :])
                                nc.tensor.matmul(
                                    psd[:], (ones_sb[:]), (e_sb[:]),
                                    start=(kt == 0), stop=(kt == nkt - 1))
                                nc.tensor.matmul(
                                    pso[:],
                                    (v_sb[:, kt, h * C.DV:(h + 1) * C.DV]),
                                    (e_sb[:]),
                                    start=(kt == 0), stop=(kt == nkt - 1))
                            rec = d_pool.tile([P, C.QT], F32, tag="rec")
                            nc.vector.reciprocal(rec[:], psd[:])
                            nc.vector.tensor_mul(ao_sb[:, h, :], pso[:], rec[:])
                    # w_o for this q tile
                    for qs in range(C.QT // P):
                        for ot in range(C.HID // 512):
                            psw = ps_w.tile([P, 512], F32, tag="psw")
                            for h in range(C.NH_G):
                                nc.tensor.matmul(
                                    psw[:],
                                    (ao_sb[:, h, qs * P:(qs + 1) * P]),
                                    (wo_sb[:, h, ot * 512:(ot + 1) * 512]),
                                    start=(h == 0), stop=(h == C.NH_G - 1))
                            oev = oev_pool.tile([P, 512], F32, tag="oev")
                            nc.vector.tensor_copy(oev[:], psw[:])
                            nc.sync.dma_start(
                                out=outp[q0 + qs * P:q0 + (qs + 1) * P,
                                         ot * 512:(ot + 1) * 512],
                                in_=oev[:])

    nc.compile()
    return nc


def rope_tables(C: Cfg):
    """cos2/ssin2 [128, S]: two stacked 64-row blocks (head pairs share)."""
    inv = 1.0 / (10000.0 ** (np.arange(0, C.DR, 2, dtype=np.float64) / C.DR))
    freqs = np.arange(C.S, dtype=np.float64)[:, None] * inv[None, :]  # [S, 32]
    emb = np.concatenate([freqs, freqs], axis=1)  # [S, 64]
    cos = np.cos(emb).T.astype(np.float32)   # [64, S]
    sin = np.sin(emb).T.astype(np.float32)
    ssin = sin.copy()
    ssin[: C.DR // 2] = -ssin[: C.DR // 2]
    cos2 = np.concatenate([cos, cos], axis=0)     # [128, S]
    ssin2 = np.concatenate([ssin, ssin], axis=0)
    return np.ascontiguousarray(cos2), np.ascontiguousarray(ssin2)


def host_inputs(C: Cfg, inputs: dict, core: int):
    """Build the per-core input map from full inputs."""
    NH = inputs["w_q_nope"].shape[1] // C.DN
    groups = NH // C.NH_G
    b = core // groups
    g = core % groups
    hs = slice(g * C.NH_G, (g + 1) * C.NH_G)

    f32 = lambda x: np.ascontiguousarray(np.asarray(x, dtype=np.float32))
    hT = f32(inputs["hidden_states"][b].T)
    w_a = f32(np.concatenate([inputs["w_q_a"], inputs["w_kv_a"]], axis=1))
    w_qbn = f32(inputs["w_q_nope"].reshape(C.QLR, NH, C.DN)[:, hs].reshape(C.QLR, -1))
    w_qbr = f32(inputs["w_q_rope"].reshape(C.QLR, NH, C.DR)[:, hs].reshape(C.QLR, -1))
    w_kbn = f32(inputs["w_k_nope"].reshape(C.KVLR, NH, C.DN)[:, hs].reshape(C.KVLR, -1))
    w_kbr = f32(inputs["w_k_rope"].reshape(C.KVLR, NH, C.DR)[:, hs].reshape(C.KVLR, -1))
    w_vb = f32(inputs["w_v"].reshape(C.KVLR, NH, C.DV)[:, hs].reshape(C.KVLR, -1))
    w_ob = f32(inputs["w_o"].reshape(NH, C.DV, C.HID)[hs].reshape(-1, C.HID))
    cos2, ssin2 = rope_tables(C)
    cm = np.asarray(inputs["causal_mask"])[0, 0]
    # diagonal-block mask in [k, q] orientation, taken from the actual input
    dmask = np.ascontiguousarray(cm[-C.QT:, -C.QT:].T.astype(np.float32))
    return {
        "hT": hT, "w_a": w_a, "w_qbn": w_qbn, "w_qbr": w_qbr,
        "w_kbn": w_kbn, "w_kbr": w_kbr, "w_vb": w_vb, "w_ob": w_ob,
        "cos2": cos2, "ssin2": ssin2, "dmask": dmask,
    }


_NC_CACHE = {}


def kernel(**inputs) -> np.ndarray:
    from concourse.bass_utils import run_bass_kernel_spmd

    C = CFG
    key = "full"
    if key not in _NC_CACHE:
        _NC_CACHE[key] = build_nc(C)
    nc = _NC_CACHE[key]

    in_maps = [host_inputs(C, inputs, c) for c in range(8)]
    res = run_bass_kernel_spmd(nc, in_maps, core_ids=list(range(8)))

    B = inputs["hidden_states"].shape[0]
    groups = 8 // B
    out = np.zeros((B, C.S, C.HID), dtype=np.float32)
    for c in range(8):
        out[c // groups] += res.results[c]["outp"]
    return out


# revision 7
# speedup vs baseline: 187.7652x; 187.7652x over previous
"""DeepseekV3 MLA attention kernel for 8 Trainium2 NeuronCores.

Sharding: 2-way data-parallel over batch x 4-way tensor-parallel over heads.
Core c handles batch b = c // 4 and heads [4*(c%4) .. 4*(c%4)+4).

Per core:
  phase 1: qT = (hidden @ (w_q_a @ w_q_b_g)).T  -- LoRA A*B fused on host
           (RoPE applied on the rope rows), kv_c = hidden @ w_kv_a
  phase 2: kT/v head projections from kv_c (RoPE on k rope rows)
  phase 3: causal attention (scoresT layout, max-free softmax with
           ones-matmul denominators) + this head-group's slice of w_o.
Host sums the 4 partial outputs per batch.

All matmuls run as float32r (fp32 storage, fast PE mode).
"""

from contextlib import ExitStack
from dataclasses import dataclass

import numpy as np

import concourse.bacc as bacc
import concourse.mybir as mybir
import concourse.tile as tile

F32 = mybir.dt.float32
F32R = mybir.dt.float32r


@dataclass(frozen=True)
class Cfg:
    S: int = 2048          # sequence length (per batch)
    HID: int = 2048        # hidden dim
    QLR: int = 1536        # q lora rank (host-side only)
    KVLR: int = 512        # kv lora rank
    NH_G: int = 4          # heads per core
    DN: int = 128          # nope dim
    DR: int = 64           # rope dim
    DV: int = 128          # v head dim
    S1T: int = 512         # phase-1 s-block width
    S2T: int = 512         # phase-2 s-tile width
    QT: int = 512          # attention q-tile width

    @property
    def QFN(self):
        return self.NH_G * self.DN      # fused q nope cols

    @property
    def QFR(self):
        return (self.NH_G // 2) * 128   # fused q rope cols (pair-packed)

    @property
    def SCALE(self):
        return 1.0 / float(np.sqrt(self.DN + self.DR))


CFG = Cfg()


def build_nc(C: Cfg, reps: int = 1):
    nc = bacc.Bacc("TRN2", target_bir_lowering=False, debug=False, num_devices=8)
    P = 128
    HO = C.HID // P
    NS1 = C.S // C.S1T
    NS2 = C.S // C.S2T
    KVC = C.KVLR // P
    NPAIR = C.NH_G // 2
    NQT = C.S // C.QT
    NDIAG = C.QT // P
    NVS = C.S // P
    NQN = C.QFN // P
    NOT = C.HID // 512

    # ---- kernel I/O ----
    hT = nc.dram_tensor("hT", [C.HID, C.S], F32R, kind="ExternalInput").ap()
    w_qf = nc.dram_tensor("w_qf", [C.HID, C.QFN + C.QFR], F32R, kind="ExternalInput").ap()
    w_kva = nc.dram_tensor("w_kva", [C.HID, C.KVLR], F32R, kind="ExternalInput").ap()
    w_kbn = nc.dram_tensor("w_kbn", [C.KVLR, C.NH_G * C.DN], F32R, kind="ExternalInput").ap()
    w_kbr = nc.dram_tensor("w_kbr", [C.KVLR, C.NH_G * C.DR], F32R, kind="ExternalInput").ap()
    w_vb = nc.dram_tensor("w_vb", [C.KVLR, C.NH_G * C.DV], F32R, kind="ExternalInput").ap()
    w_ob = nc.dram_tensor("w_ob", [C.NH_G * C.DV, C.HID], F32R, kind="ExternalInput").ap()
    cos2 = nc.dram_tensor("cos2", [P, C.S], F32, kind="ExternalInput").ap()
    ssin2 = nc.dram_tensor("ssin2", [P, C.S], F32, kind="ExternalInput").ap()
    dmask = nc.dram_tensor("dmask", [C.QT, C.QT], F32, kind="ExternalInput").ap()
    outp = nc.dram_tensor("outp", [C.S, C.HID], F32, kind="ExternalOutput").ap()

    # ---- DRAM scratch ----
    kv_d = nc.dram_tensor("kv_scr", [C.KVLR, C.S], F32R).ap()
    qTn_d = nc.dram_tensor("qTn_scr", [C.QFN, C.S], F32R).ap()
    qTr_d = nc.dram_tensor("qTr_scr", [C.QFR, C.S], F32R).ap()

    with tile.TileContext(nc) as tc:
        for rep in range(reps):
            with ExitStack() as tctx:
                tab_pool = tctx.enter_context(tc.tile_pool(name=f"tab{rep}", bufs=1))
                cos_sb = tab_pool.tile([P, C.S], F32)
                ssin_sb = tab_pool.tile([P, C.S], F32)
                nc.sync.dma_start(out=cos_sb[:], in_=cos2)
                nc.sync.dma_start(out=ssin_sb[:], in_=ssin2)

                def rope_evict(rp_pool, ps, dst_ap, s0, slen):
                    """psum [128, slen] of rope rows (head pair) -> roped into dst."""
                    tmp = rp_pool.tile([P, slen], F32R, tag="rope_tmp")
                    nc.vector.tensor_copy(tmp[:], ps[:])
                    qs = rp_pool.tile([P, slen], F32R, tag="rope_qs")
                    for g in range(4):
                        nc.sync.dma_start(
                            out=qs[(g ^ 1) * 32:(g ^ 1) * 32 + 32, :],
                            in_=tmp[g * 32:(g + 1) * 32, :])
                    m1 = rp_pool.tile([P, slen], F32R, tag="rope_m1")
                    nc.vector.tensor_mul(m1[:], tmp[:], cos_sb[:, s0:s0 + slen])
                    nc.vector.tensor_mul(qs[:], qs[:], ssin_sb[:, s0:s0 + slen])
                    nc.vector.tensor_add(dst_ap, m1[:], qs[:])

                # ===== Phase 1: fused q projection + kv latent, from hidden =====
                with ExitStack() as ctx:
                    wq_pool = ctx.enter_context(tc.tile_pool(name=f"wqf{rep}", bufs=1))
                    ht_pool = ctx.enter_context(tc.tile_pool(name=f"ht{rep}", bufs=2))
                    ev_pool = ctx.enter_context(tc.tile_pool(name=f"s1ev{rep}", bufs=4))
                    rp_pool = ctx.enter_context(tc.tile_pool(name=f"s1rp{rep}", bufs=3))
                    ps_pool = ctx.enter_context(
                        tc.tile_pool(name=f"s1ps{rep}", bufs=4, space="PSUM"))

                    wqf_sb = wq_pool.tile([P, HO, C.QFN + C.QFR], F32R)
                    wkva_sb = wq_pool.tile([P, HO, C.KVLR], F32R)
                    wqf_r = w_qf.rearrange("(ho hi) c -> hi ho c", hi=P)
                    wkva_r = w_kva.rearrange("(ho hi) c -> hi ho c", hi=P)
                    NCOL = (C.QFN + C.QFR) // P
                    for cc in range(NCOL):
                        nc.sync.dma_start(
                            out=wqf_sb[:, :, cc * P:(cc + 1) * P],
                            in_=wqf_r[:, :, cc * P:(cc + 1) * P])
                    for cc in range(KVC):
                        nc.sync.dma_start(
                            out=wkva_sb[:, :, cc * P:(cc + 1) * P],
                            in_=wkva_r[:, :, cc * P:(cc + 1) * P])
                    hT_r = hT.rearrange("(ho hi) s -> hi ho s", hi=P)
                    for st in range(NS1):
                        s0 = st * C.S1T
                        ht_sb = ht_pool.tile([P, HO, C.S1T], F32R)
                        nc.sync.dma_start(out=ht_sb[:], in_=hT_r[:, :, s0:s0 + C.S1T])

                        def accum(lhs_sb, col0):
                            ps = ps_pool.tile([P, C.S1T], F32, tag="ps1")
                            for h in range(HO):
                                nc.tensor.matmul(
                                    ps[:], lhs_sb[:, h, col0:col0 + P],
                                    ht_sb[:, h, :],
                                    start=(h == 0), stop=(h == HO - 1))
                            return ps

                        for t in range(NQN):
                            ps = accum(wqf_sb, t * P)
                            ev = ev_pool.tile([P, C.S1T], F32R)
                            nc.vector.tensor_copy(ev[:], ps[:])
                            nc.sync.dma_start(
                                out=qTn_d[t * P:(t + 1) * P, s0:s0 + C.S1T], in_=ev[:])
                        for pr in range(NPAIR):
                            ps = accum(wqf_sb, C.QFN + pr * P)
                            ev = ev_pool.tile([P, C.S1T], F32R)
                            rope_evict(rp_pool, ps, ev[:], s0, C.S1T)
                            nc.sync.dma_start(
                                out=qTr_d[pr * P:(pr + 1) * P, s0:s0 + C.S1T], in_=ev[:])
                        for cc in range(KVC):
                            ps = accum(wkva_sb, cc * P)
                            ev = ev_pool.tile([P, C.S1T], F32R)
                            nc.vector.tensor_copy(ev[:], ps[:])
                            nc.sync.dma_start(
                                out=kv_d[cc * P:(cc + 1) * P, s0:s0 + C.S1T], in_=ev[:])

                # ===== Phase 2: kT / v projections (resident for attention) =====
                per_pool = tctx.enter_context(tc.tile_pool(name=f"persist{rep}", bufs=1))
                kTn_sb = per_pool.tile([P, C.NH_G, C.S], F32R)
                kTr_sb = per_pool.tile([P, NPAIR, C.S], F32R)
                v_sb = per_pool.tile([P, NVS, C.NH_G * C.DV], F32R)
                with ExitStack() as ctx:
                    wk_pool = ctx.enter_context(tc.tile_pool(name=f"wk{rep}", bufs=1))
                    kv_pool = ctx.enter_context(tc.tile_pool(name=f"kvs{rep}", bufs=2))
                    rp_pool = ctx.enter_context(tc.tile_pool(name=f"s2rp{rep}", bufs=3))
                    ps_pool = ctx.enter_context(
                        tc.tile_pool(name=f"s2ps{rep}", bufs=4, space="PSUM"))

                    wkn_sb = wk_pool.tile([P, KVC, C.NH_G * C.DN], F32R)
                    wkr_sb = wk_pool.tile([P, KVC, C.NH_G * C.DR], F32R)
                    wv_sb = wk_pool.tile([P, KVC, C.NH_G * C.DV], F32R)
                    nc.sync.dma_start(
                        out=wkn_sb[:], in_=w_kbn.rearrange("(co ci) m -> ci co m", ci=P))
                    nc.sync.dma_start(
                        out=wkr_sb[:], in_=w_kbr.rearrange("(co ci) m -> ci co m", ci=P))
                    nc.sync.dma_start(
                        out=wv_sb[:], in_=w_vb.rearrange("(co ci) m -> ci co m", ci=P))
                    kv_r = kv_d.rearrange("(co ci) s -> ci co s", ci=P)
                    for st in range(NS2):
                        s0 = st * C.S2T
                        kv_sb = kv_pool.tile([P, KVC, C.S2T], F32R)
                        nc.sync.dma_start(out=kv_sb[:], in_=kv_r[:, :, s0:s0 + C.S2T])
                        for h in range(C.NH_G):
                            ps = ps_pool.tile([P, C.S2T], F32, tag="ps2")
                            for cch in range(KVC):
                                nc.tensor.matmul(
                                    ps[:], wkn_sb[:, cch, h * C.DN:(h + 1) * C.DN],
                                    kv_sb[:, cch, :],
                                    start=(cch == 0), stop=(cch == KVC - 1))
                            nc.vector.tensor_copy(kTn_sb[:, h, s0:s0 + C.S2T], ps[:])
                        for pr in range(NPAIR):
                            ps = ps_pool.tile([P, C.S2T], F32, tag="ps2")
                            for cch in range(KVC):
                                nc.tensor.matmul(
                                    ps[:], wkr_sb[:, cch, pr * P:(pr + 1) * P],
                                    kv_sb[:, cch, :],
                                    start=(cch == 0), stop=(cch == KVC - 1))
                            rope_evict(rp_pool, ps, kTr_sb[:, pr, s0:s0 + C.S2T],
                                       s0, C.S2T)
                        for ssub in range(C.S2T // P):
                            vs = (s0 + ssub * P) // P
                            ps = ps_pool.tile([P, C.NH_G * C.DV], F32, tag="ps2")
                            for cch in range(KVC):
                                nc.tensor.matmul(
                                    ps[:], kv_sb[:, cch, ssub * P:(ssub + 1) * P],
                                    wv_sb[:, cch, :],
                                    start=(cch == 0), stop=(cch == KVC - 1))
                            nc.vector.tensor_copy(v_sb[:, vs, :], ps[:])

                # ================= Phase 3: attention + w_o =================
                with ExitStack() as ctx:
                    const_pool = ctx.enter_context(tc.tile_pool(name=f"ac{rep}", bufs=1))
                    qn_pool = ctx.enter_context(tc.tile_pool(name=f"aqn{rep}", bufs=2))
                    qr_pool = ctx.enter_context(tc.tile_pool(name=f"aqr{rep}", bufs=2))
                    e_pool = ctx.enter_context(tc.tile_pool(name=f"ae{rep}", bufs=4))
                    d_pool = ctx.enter_context(tc.tile_pool(name=f"ad{rep}", bufs=2))
                    ao_pool = ctx.enter_context(tc.tile_pool(name=f"aao{rep}", bufs=2))
                    oev_pool = ctx.enter_context(tc.tile_pool(name=f"aoe{rep}", bufs=4))
                    ps_s = ctx.enter_context(
                        tc.tile_pool(name=f"apss{rep}", bufs=3, space="PSUM"))
                    ps_d = ctx.enter_context(
                        tc.tile_pool(name=f"apsd{rep}", bufs=1, space="PSUM"))
                    ps_o = ctx.enter_context(
                        tc.tile_pool(name=f"apso{rep}", bufs=2, space="PSUM"))
                    ps_w = ctx.enter_context(
                        tc.tile_pool(name=f"apsw{rep}", bufs=2, space="PSUM"))

                    ones_f = const_pool.tile([P, P], F32)
                    ones_sb = const_pool.tile([P, P], F32R)
                    nc.vector.memset(ones_f[:], 1.0)
                    nc.vector.tensor_copy(ones_sb[:], ones_f[:])
                    dm_sb = const_pool.tile([P, NDIAG, C.QT], F32)
                    nc.sync.dma_start(
                        out=dm_sb[:], in_=dmask.rearrange("(j ki) q -> ki j q", ki=P))
                    wo_sb = const_pool.tile([P, C.NH_G, C.HID], F32R)
                    nc.sync.dma_start(
                        out=wo_sb[:], in_=w_ob.rearrange("(h d) o -> d h o", d=P))

                    for qt in range(NQT):
                        q0 = qt * C.QT
                        nkt = (qt + 1) * C.QT // P
                        ao_sb = ao_pool.tile([P, C.NH_G, C.QT], F32R, tag="ao")
                        for pr in range(NPAIR):
                            qr_sb = qr_pool.tile([P, C.QT], F32R, tag="qr")
                            nc.sync.dma_start(
                                out=qr_sb[:],
                                in_=qTr_d[pr * P:(pr + 1) * P, q0:q0 + C.QT])
                            for hh in range(2):
                                h = pr * 2 + hh
                                qn_sb = qn_pool.tile([P, C.QT], F32R, tag="qn")
                                nc.sync.dma_start(
                                    out=qn_sb[:],
                                    in_=qTn_d[h * C.DN:(h + 1) * C.DN, q0:q0 + C.QT])
                                rsl = slice(hh * C.DR, (hh + 1) * C.DR)
                                psd = ps_d.tile([P, C.QT], F32, tag="psd")
                                pso = ps_o.tile([P, C.QT], F32, tag="pso")
                                for kt in range(nkt):
                                    k0 = kt * P
                                    pss = ps_s.tile([P, C.QT], F32, tag="pss")
                                    nc.tensor.matmul(
                                        pss[:], kTn_sb[:, h, k0:k0 + P], qn_sb[:],
                                        start=True, stop=False)
                                    nc.tensor.matmul(
                                        pss[:], kTr_sb[rsl, pr, k0:k0 + P],
                                        qr_sb[rsl, :],
                                        start=False, stop=True)
                                    e_sb = e_pool.tile([P, C.QT], F32R, tag="e")
                                    nc.scalar.activation(
                                        e_sb[:], pss[:],
                                        mybir.ActivationFunctionType.Exp,
                                        scale=C.SCALE)
                                    j = kt - qt * NDIAG
                                    if j >= 0:
                                        nc.vector.tensor_mul(
                                            e_sb[:], e_sb[:], dm_sb[:, j, :])
                                    nc.tensor.matmul(
                                        psd[:], ones_sb[:], e_sb[:],
                                        start=(kt == 0), stop=(kt == nkt - 1))
                                    nc.tensor.matmul(
                                        pso[:], v_sb[:, kt, h * C.DV:(h + 1) * C.DV],
                                        e_sb[:],
                                        start=(kt == 0), stop=(kt == nkt - 1))
                                rec = d_pool.tile([P, C.QT], F32, tag="rec")
                                nc.vector.reciprocal(rec[:], psd[:])
                                nc.vector.tensor_mul(ao_sb[:, h, :], pso[:], rec[:])
                        for qs in range(C.QT // P):
                            for ot in range(NOT):
                                psw = ps_w.tile([P, 512], F32, tag="psw")
                                for h in range(C.NH_G):
                                    nc.tensor.matmul(
                                        psw[:], ao_sb[:, h, qs * P:(qs + 1) * P],
                                        wo_sb[:, h, ot * 512:(ot + 1) * 512],
                                        start=(h == 0), stop=(h == C.NH_G - 1))
                                oev = oev_pool.tile([P, 512], F32)
                                nc.vector.tensor_copy(oev[:], psw[:])
                                nc.sync.dma_start(
                                    out=outp[q0 + qs * P:q0 + (qs + 1) * P,
                                             ot * 512:(ot + 1) * 512],
                                    in_=oev[:])

    nc.compile()
    return nc


def rope_tables(C: Cfg):
    """cos2/ssin2 [128, S]: two stacked 64-row blocks (head pairs share)."""
    inv = 1.0 / (10000.0 ** (np.arange(0, C.DR, 2, dtype=np.float64) / C.DR))
    freqs = np.arange(C.S, dtype=np.float64)[:, None] * inv[None, :]  # [S, 32]
    emb = np.concatenate([freqs, freqs], axis=1)  # [S, 64]
    cos = np.cos(emb).T.astype(np.float32)   # [64, S]
    sin = np.sin(emb).T.astype(np.float32)
    ssin = sin.copy()
    ssin[: C.DR // 2] = -ssin[: C.DR // 2]
    cos2 = np.concatenate([cos, cos], axis=0)     # [128, S]
    ssin2 = np.concatenate([ssin, ssin], axis=0)
    return np.ascontiguousarray(cos2), np.ascontiguousarray(ssin2)


def host_inputs(C: Cfg, inputs: dict, core: int):
    """Build the per-core input map from full inputs."""
    NH = inputs["w_q_nope"].shape[1] // C.DN
    groups = NH // C.NH_G
    b = core // groups
    g = core % groups
    hs = slice(g * C.NH_G, (g + 1) * C.NH_G)

    f32 = lambda x: np.ascontiguousarray(np.asarray(x, dtype=np.float32))
    hT = f32(inputs["hidden_states"][b].T)
    w_q_a = np.asarray(inputs["w_q_a"], dtype=np.float32)
    w_qbn = f32(inputs["w_q_nope"].reshape(C.QLR, NH, C.DN)[:, hs].reshape(C.QLR, -1))
    w_qbr = f32(inputs["w_q_rope"].reshape(C.QLR, NH, C.DR)[:, hs].reshape(C.QLR, -1))
    w_qf = f32(np.concatenate([w_q_a @ w_qbn, w_q_a @ w_qbr], axis=1))
    w_kva = f32(inputs["w_kv_a"])
    w_kbn = f32(inputs["w_k_nope"].reshape(C.KVLR, NH, C.DN)[:, hs].reshape(C.KVLR, -1))
    w_kbr = f32(inputs["w_k_rope"].reshape(C.KVLR, NH, C.DR)[:, hs].reshape(C.KVLR, -1))
    w_vb = f32(inputs["w_v"].reshape(C.KVLR, NH, C.DV)[:, hs].reshape(C.KVLR, -1))
    w_ob = f32(inputs["w_o"].reshape(NH, C.DV, C.HID)[hs].reshape(-1, C.HID))
    cos2, ssin2 = rope_tables(C)
    cm = np.asarray(inputs["causal_mask"])[0, 0]
    dmask = np.ascontiguousarray(cm[-C.QT:, -C.QT:].T.astype(np.float32))
    return {
        "hT": hT, "w_qf": w_qf, "w_kva": w_kva,
        "w_kbn": w_kbn, "w_kbr": w_kbr, "w_vb": w_vb, "w_ob": w_ob,
        "cos2": cos2, "ssin2": ssin2, "dmask": dmask,
    }


_NC_CACHE = {}


def kernel(**inputs) -> np.ndarray:
    from concourse.bass_utils import run_bass_kernel_spmd

    C = CFG
    if "nc" not in _NC_CACHE:
        _NC_CACHE["nc"] = build_nc(C)
    nc = _NC_CACHE["nc"]

    in_maps = [host_inputs(C, inputs, c) for c in range(8)]
    res = run_bass_kernel_spmd(nc, in_maps, core_ids=list(range(8)))

    B = inputs["hidden_states"].shape[0]
    groups = 8 // B
    out = np.zeros((B, C.S, C.HID), dtype=np.float32)
    for c in range(8):
        out[c // groups] += res.results[c]["outp"]
    return out
